# revision 9
# baseline (speedup 1.0000x reference)
"""Binary position embedding kernel for Trainium2 (8 NeuronCores, SPMD).

out[t, :] = sum_{b : bit b of x[t] set} emb[b, :]   ==   mask(x) @ emb

Strategy (data-parallel over tokens, per the sharding hint):
  - Flatten x (4, 8192) -> (32768,), shard 4096 tokens per core; the
    (tiny) emb table is replicated.  Each core computes its (4096, 1024)
    output slab; host concatenates.
  - The output is written as bf16 and upcast to f32 on the host: the
    kernel is memory-bound on output writes, and bf16 halves the 16 MiB
    of f32 traffic per core while its <=2^-9 relative rounding error is
    ~10x under the 2e-2 gate.  The measured per-core DMA write plateau
    is ~325 GB/s (HBM share; a second HWDGE queue adds only ~3%), so
    8 MiB of bf16 writes floor the kernel at ~26 us.
  - emb is split hi/lo into bf16 (hi = bf16(emb), lo = bf16(emb - hi),
    |err| ~2^-16 relative) stacked into a KP=64 partition tile (hi at
    partition 0, lo at 32): one K=64 bf16 matmul computes mask@(hi+lo)
    with f32 PSUM accumulation.
  - Bit b of integer x via pure-f32 arithmetic on DVE:
    t = (x + 0.25) * 2^-(b+1);  r = (t + 2^23) - 2^23  (RNE round,
    tie-free thanks to the +0.25);  bit = (t < r).  Rows with pw=0
    yield exactly-zero mask rows, so unused partitions are inert.
  - Per 128-token j-tile: 2 matmuls into a [128, 1024] 2-bank PSUM
    tile, one PSUM->SBUF bf16-converting copy (split DVE/ACT per COPY_W
    so both engines stay under the DMA wall; Pool cannot read PSUM),
    one 256 KiB output DMA, alternating between the SP and ACT HWDGE
    queues.
  - x (sent as exact f32) is partition-broadcast by the Pool engine;
    the first PW_XB0_TOK tokens ride pre-broadcast inside the pw DMA so
    the first mask op waits on nothing else.
"""

import sys

import numpy as np

if "/opt/trn_rl_repo" not in sys.path:
    sys.path.insert(0, "/opt/trn_rl_repo")

N_BITS = 13
D_MODEL = 1024
N_CORES = 8
TOKENS = 4 * 8192
TOK_PER_CORE = TOKENS // N_CORES  # 4096

# Contraction layout: hi(emb) in partitions [0,13), lo in [32,45) of a
# 64-partition tile (engine writes must start at partition 0/32/64/96).
KP = 64
LO_OFF = 32
MMT = 128  # tokens per matmul (output partition dim)
N_J = TOK_PER_CORE // MMT  # 32 j-tiles

GROUPS = [256, 256] + [512] * 7  # token groups; sum = TOK_PER_CORE
PW_XB0_TOK = 512  # leading tokens whose pre-broadcast x rides in the pw DMA
# Pool cannot access PSUM on TRN2 (verifier), and its partition_broadcast
# only lands at a base-0 destination.  Engine assignment keeps each engine
# on ONE job: Pool broadcasts x, DVE computes the mask chain, ACT is the
# sole PSUM reader (all 32 PSUM->SBUF bf16 copies: concurrent DVE+ACT PSUM
# reads were measured to serialize, 48us vs 31us), SP issues all out DMAs.
COPY_W = (0, 32, 0)  # j-tile copy split (DVE, ACT, Pool)
DMA_PAT = "s"  # per-j output DMA queue: s=SP(sync) a=ACT(scalar)
PSUM_BUFS = 3  # [128,1024] 2-bank tiles
OUTP_BUFS = 6
MASKP_BUFS = 3
STAGGERED_RESET = False  # benchmark loop back-edge mode

_CACHE = {}
last_results = None  # BassKernelResults of the most recent run (for test.py)


def _copy_engines():
    """Spread COPY_W copies per engine evenly over the N_J j-tiles."""
    used = [0, 0, 0]
    out = []
    for j in range(N_J):
        deficits = [COPY_W[e] * (j + 1) / N_J - used[e] for e in range(3)]
        e = max(range(3), key=lambda i: deficits[i])
        used[e] += 1
        out.append(e)
    return out


def _build_module(loop_reps=None):
    """Build the per-core Bass module.

    loop_reps: if set, wrap the whole pipeline in a tc.For_i repetition
    loop (benchmark-only; ~2us back-edge per iteration).
    """
    import concourse.bacc as bacc
    import concourse.mybir as mybir
    import concourse.tile as tile
    from contextlib import ExitStack

    f32 = mybir.dt.float32
    bf16 = mybir.dt.bfloat16

    nc = bacc.Bacc("TRN2", target_bir_lowering=False)

    assert sum(GROUPS) == TOK_PER_CORE
    x_d = nc.dram_tensor("x", [1, TOK_PER_CORE], f32, kind="ExternalInput")
    embhl_d = nc.dram_tensor("embhl", [KP, D_MODEL], bf16, kind="ExternalInput")
    pw_cols = 1 + PW_XB0_TOK
    pw_d = nc.dram_tensor("pw", [KP, pw_cols], f32, kind="ExternalInput")
    out_d = nc.dram_tensor("out", [TOK_PER_CORE, D_MODEL], bf16, kind="ExternalOutput")

    # DRAM view [p, j, d]: token index = j*MMT + p  (j counts MMT tiles)
    out_pjd = out_d.rearrange("(j p) d -> p j d", p=MMT)

    copy_eng = _copy_engines()

    with ExitStack() as ctx:
        tc = ctx.enter_context(tile.TileContext(nc))
        if loop_reps is not None:
            ctx.enter_context(
                tc.For_i(0, loop_reps, 1, staggered_reset=STAGGERED_RESET)
            )
        const = ctx.enter_context(tc.tile_pool(name="const", bufs=1))
        maskp = ctx.enter_context(tc.tile_pool(name="maskp", bufs=MASKP_BUFS))
        psum = ctx.enter_context(tc.tile_pool(name="psum", bufs=PSUM_BUFS, space="PSUM"))
        outp = ctx.enter_context(tc.tile_pool(name="outp", bufs=OUTP_BUFS))

        # --- constants ---  (pw2, which also carries the pre-broadcast x for
        # the first PW_XB0_TOK tokens, goes first: it gates the first mask op)
        pw2 = const.tile([KP, pw_cols], f32)
        nc.sync.dma_start(pw2[:], pw_d[:])
        pw = pw2[:, 0:1]
        emb_hl = const.tile([KP, D_MODEL], bf16)
        nc.scalar.dma_start(emb_hl[:], embhl_d[:])
        x_sb = const.tile([1, TOK_PER_CORE], f32)
        nc.sync.dma_start(x_sb[:], x_d[:])

        # PE warm-up: input-independent dummy matmuls keep the PE busy for
        # the first ~4us so the HAM throttle is at full rate when the real
        # matmuls arrive (cold PE runs at half rate for ~3-4us).
        warm_l = const.tile([KP, MMT], bf16)
        warm_r = const.tile([KP, 512], bf16)
        nc.gpsimd.memset(warm_l[:], 0.0)
        nc.gpsimd.memset(warm_r[:], 0.0)
        warmp = ctx.enter_context(tc.tile_pool(name="warmp", bufs=1, space="PSUM"))
        warm_ps = warmp.tile([MMT, 512], f32, tag="warm")
        for _ in range(5):
            nc.tensor.matmul(warm_ps[:], warm_l[:], warm_r[:], start=True, stop=True)
        # ACT warm-up: force the activation-function table load (~1.3us)
        # off the first real copy's critical path
        warm_act = const.tile([KP, 8], bf16)
        nc.scalar.copy(warm_act[:], warm_l[:, 0:8])

        # --- main loop ---
        tok0 = 0
        jg = 0  # global j-tile index
        for gtok in GROUPS:
            n_mmt = gtok // MMT

            if tok0 + gtok <= PW_XB0_TOK:
                xb_ap = pw2[:, 1 + tok0 : 1 + tok0 + gtok]
            else:
                xb = maskp.tile([KP, gtok], f32, tag="xb")
                nc.gpsimd.partition_broadcast(
                    xb[:], x_sb[0:1, tok0 : tok0 + gtok]
                )
                xb_ap = xb[:]

            t = maskp.tile([KP, gtok], f32, tag="t")
            nc.vector.tensor_scalar(
                out=t[:],
                in0=xb_ap,
                scalar1=0.25,
                scalar2=pw[:],
                op0=mybir.AluOpType.add,
                op1=mybir.AluOpType.mult,
            )
            r = maskp.tile([KP, gtok], f32, tag="r")
            nc.vector.tensor_scalar(
                out=r[:],
                in0=t[:],
                scalar1=float(2**23),
                scalar2=float(2**23),
                op0=mybir.AluOpType.add,
                op1=mybir.AluOpType.subtract,
            )
            mask = maskp.tile([KP, gtok], bf16, tag="mask")
            nc.vector.tensor_tensor(
                out=mask[:], in0=t[:], in1=r[:], op=mybir.AluOpType.is_lt
            )

            for jc in range(n_mmt):
                ps = psum.tile([MMT, D_MODEL], f32, tag="ps")
                for h in range(2):
                    nc.tensor.matmul(
                        ps[:, h * 512 : (h + 1) * 512],
                        mask[:, jc * MMT : (jc + 1) * MMT],
                        emb_hl[:, h * 512 : (h + 1) * 512],
                        start=True,
                        stop=True,
                    )
                ob = outp.tile([MMT, D_MODEL], bf16, tag="ob")
                ce = copy_eng[jg]
                if ce == 0:
                    nc.vector.tensor_copy(ob[:], ps[:])
                elif ce == 1:
                    nc.scalar.copy(ob[:], ps[:])
                else:
                    nc.gpsimd.tensor_copy(ob[:], ps[:])
                dq = DMA_PAT[jg % len(DMA_PAT)]
                deng = nc.sync if dq == "s" else nc.scalar
                deng.dma_start(out_pjd[:, jg], ob[:])
                jg += 1
            tok0 += gtok

    nc.compile()
    return nc


def _get_module():
    if "nc" not in _CACHE:
        _CACHE["nc"] = _build_module()
    return _CACHE["nc"]


def _make_consts(emb):
    """Host-precomputed constant tables: per-partition bit scales (pw) and
    the hi/lo bf16 split of emb stacked at partitions 0/32 and 64/96."""
    import ml_dtypes

    pw = np.zeros((KP, 1), dtype=np.float32)
    bits = np.arange(N_BITS, dtype=np.float64)
    for off in (0, LO_OFF):
        pw[off : off + N_BITS, 0] = 2.0 ** -(bits + 1.0)

    emb = np.asarray(emb, dtype=np.float32)
    hi = emb.astype(ml_dtypes.bfloat16)
    lo = (emb - hi.astype(np.float32)).astype(ml_dtypes.bfloat16)
    embhl = np.zeros((KP, D_MODEL), dtype=ml_dtypes.bfloat16)
    embhl[0:N_BITS] = hi
    embhl[LO_OFF : LO_OFF + N_BITS] = lo
    return pw, embhl


def _make_in_maps(x_f32, emb):
    """Per-core input dicts: x shard, const tables, per-shard pw with the
    first PW_XB0_TOK tokens pre-broadcast in packed [128, n] layout."""
    pw, embhl = _make_consts(emb)
    in_maps = []
    for c in range(N_CORES):
        shard = x_f32[c * TOK_PER_CORE : (c + 1) * TOK_PER_CORE].reshape(
            1, TOK_PER_CORE
        )
        pw_c = np.ascontiguousarray(
            np.concatenate(
                [pw, np.broadcast_to(shard[0, 0:PW_XB0_TOK], (KP, PW_XB0_TOK))],
                axis=1,
            ),
            dtype=np.float32,
        )
        in_maps.append(
            {"x": np.ascontiguousarray(shard), "embhl": embhl, "pw": pw_c}
        )
    return in_maps


def kernel(x, emb):
    global last_results
    from concourse.bass_utils import run_bass_kernel_spmd

    x = np.asarray(x)
    emb = np.asarray(emb, dtype=np.float32)
    orig_shape = x.shape
    x_flat = x.reshape(-1)
    assert x_flat.shape[0] == TOKENS
    x_f32 = x_flat.astype(np.float32)  # values < 8192, exact in f32
    in_maps = _make_in_maps(x_f32, emb)

    nc = _get_module()
    res = run_bass_kernel_spmd(nc, in_maps, core_ids=list(range(N_CORES)))
    last_results = res

    out = np.concatenate(
        [np.asarray(res.results[c]["out"]).astype(np.float32) for c in range(N_CORES)],
        axis=0,
    )
    return out.reshape(*orig_shape, D_MODEL)
